# revision 14
# baseline (speedup 1.0000x reference)
"""Trainium2 Bass kernel for 2-layer GCN forward (Reddit-like), 8-way node-sharded.

Strategy (matches the sharding hint):
- Nodes partitioned contiguously across 8 cores (12500 each). Edges (with
  self-loops) are routed to the core owning their destination node; within a
  core they are grouped by 128-node destination tile and by source "bank"
  (h tables are gathered with int16 indices, so the 100000-row feature table
  is split into 4 banks of <32768 rows; boundaries hill-climbed to minimize
  chunk padding).
- Per core: h1 = x_shard @ W1 on PE (bf16), AllGather the 12.5k x 128 shard ->
  full h1 table in DRAM; per edge-chunk of 128 edges, dma_gather the source
  rows, and segment-sum via a norm-scaled one-hot selection matrix matmul
  accumulated in PSUM (S[e, n] = (dst_rel[e] == n) * norm[e]).
  relu(+b1), @W2, AllGather again, same aggregation, +b2, log_softmax.
- All tensors feeding the PE are bf16 (1 cycle/row vs 4 for fp32); PSUM
  accumulation stays fp32. h tables are padded to 128 cols so gather rows are
  256B (DMA gather requires elem_size and row stride to be multiples of 256B).
- Self-loop rows are kept resident in SBUF (h1keep/h2keep) instead of being
  re-loaded from DRAM; gather calls are spread over 4 SWDGE queues.
- Weight matrices are replicated; all index preprocessing is host-side numpy.
"""
import math
import numpy as np
import ml_dtypes
from contextlib import ExitStack

import concourse.bass as bass
import concourse.bacc as bacc
import concourse.tile as tile
from concourse import mybir
from concourse.bass_utils import run_bass_kernel_spmd

BF16 = ml_dtypes.bfloat16

# problem sizes (hardcoded per the contract)
N = 100000
E = 1250000
F_IN = 602
F_PAD = 640          # 5 x 128
HID = 64
HID2 = 128           # padded h-table width (256B bf16 rows)
C = 41
N_CORES = 8
NT = N // N_CORES    # 12500 nodes per core
P = 128
N_TILES = (NT + P - 1) // P          # 98 (last tile 84 nodes)
N_BANKS = 4
GROUP = 6                            # node tiles per gather call group

PROFILE = False      # set True from test harness to request an NTFF trace
SIM_MODE = False     # replace collectives with local copies (TimelineSim only)
VARIANT = "full"     # timing decomposition: "full" | "nogather" | "nocompute"
N_QUEUES = 4         # SWDGE queues to spread gather calls across (1..4)
_LAST_RESULTS = {}   # debug: profile info stash


def _tune_banks(s, core_tile):
    """Hill-climb interior bank boundaries to minimize total chunk count
    sum over (tile,bank) of ceil(max_core_count/128)."""
    def nch(bounds):
        bank = np.searchsorted(bounds, s, side="right")
        key = core_tile * N_BANKS + bank
        counts = np.bincount(key, minlength=N_CORES * N_TILES * N_BANKS)
        counts = counts.reshape(N_CORES, N_TILES, N_BANKS)
        return int(np.ceil(counts.max(axis=0) / P).sum())

    best = np.asarray([29500, 59000, 88500], dtype=np.int64)
    best_v = nch(best)
    for step in (4000, 2000, 1000, 500, 250):
        improved = True
        while improved:
            improved = False
            for i in range(len(best)):
                for sgn in (+1, -1):
                    cand = best.copy()
                    cand[i] += sgn * step
                    lo = 0 if i == 0 else int(cand[i - 1])
                    hi = N if i == len(best) - 1 else int(cand[i + 1])
                    if not (lo < cand[i] < hi):
                        continue
                    bb = [0] + [int(b) for b in cand] + [N]
                    if max(b2 - b1 for b1, b2 in zip(bb, bb[1:])) > 32767:
                        continue
                    v = nch(cand)
                    if v < best_v:
                        best, best_v = cand, v
                        improved = True
    return [0] + [int(b) for b in best] + [N], best_v


def _preprocess(x, src, dst, W1, b1, W2, b2):
    """Host-side index preprocessing and sharding. Returns in_maps + plan."""
    src = np.asarray(src).astype(np.int64).ravel()
    dst = np.asarray(dst).astype(np.int64).ravel()
    x = np.asarray(x, dtype=np.float32)
    W1 = np.asarray(W1, dtype=np.float32)
    b1 = np.asarray(b1, dtype=np.float32)
    W2 = np.asarray(W2, dtype=np.float32)
    b2 = np.asarray(b2, dtype=np.float32)

    # degrees include self-loops (reference semantics); the self-loop term
    # itself is applied on-device as a diagonal matmul, not via gathers.
    deg = (np.bincount(dst, minlength=N) + 1.0).astype(np.float32)
    dinv = (1.0 / np.sqrt(deg)).astype(np.float32)
    s = src
    d = dst
    norm = (dinv[s] * dinv[d]).astype(np.float32)

    core = d // NT
    trel = (d % NT) // P
    core_tile = core * N_TILES + trel
    bank_bounds, _ = _tune_banks(s, core_tile)
    bounds = np.asarray(bank_bounds[1:-1], dtype=np.int64)

    bank = np.searchsorted(bounds, s, side="right")
    key = (core_tile * N_BANKS + bank).astype(np.int64)
    order = np.argsort(key, kind="stable")
    s_o = s[order]
    d_o = d[order]
    n_o = norm[order]
    key_o = key[order]

    n_keys = N_CORES * N_TILES * N_BANKS
    counts = np.bincount(key_o, minlength=n_keys).reshape(N_CORES, N_TILES, N_BANKS)
    # uniform chunk counts across cores (single SPMD program)
    cols = np.ceil(counts.max(axis=0) / P).astype(np.int64)      # [N_TILES, N_BANKS]
    NCH = int(cols.sum())                                         # chunks per core

    # group boundaries for gather calls: groups of GROUP node tiles
    groups = [
        list(range(g, min(g + GROUP, N_TILES))) for g in range(0, N_TILES, GROUP)
    ]

    # re-order chunks canonically: for g: for b: for t in g: cols[t][b] chunks
    # chunk_index[t][b] = start in canonical order
    ch_index = np.zeros((N_TILES, N_BANKS), dtype=np.int64)
    call_plan = []  # per group: list of (b, ch0, n_chunks) with ch0 canonical
    group_start = []
    cursor = 0
    for g in groups:
        group_start.append(cursor)
        for b in range(N_BANKS):
            cw = int(cols[np.array(g), b].sum())
            if cw:
                call_plan.append((len(group_start) - 1, b, cursor, cw))
            for t in g:
                ch_index[t, b] = cursor
                cursor += int(cols[t, b])
    assert cursor == NCH

    # per-core edge placement
    in_maps = []
    for c in range(N_CORES):
        lo = np.searchsorted(key_o, c * N_TILES * N_BANKS)
        hi = np.searchsorted(key_o, (c + 1) * N_TILES * N_BANKS)
        ks = key_o[lo:hi] - c * N_TILES * N_BANKS       # (t * N_BANKS + b)
        # position within the (t,b) run for each edge
        run_starts = np.concatenate(
            [[0], np.cumsum(np.bincount(ks, minlength=N_TILES * N_BANKS))]
        )[:-1]
        i_local = np.arange(hi - lo) - run_starts[ks]
        chv = ch_index.ravel()[ks] + i_local // P
        pv = (i_local % P).astype(np.int64)

        erel = np.zeros((P, NCH), dtype=np.float32)
        enrm = np.zeros((P, NCH), dtype=np.float32)
        idx16 = np.zeros((16, NCH * 8), dtype=np.int16)
        drel = (d_o[lo:hi] - (c * NT + (ks // N_BANKS) * P)).astype(np.float32)
        erel[pv, chv] = drel
        enrm[pv, chv] = n_o[lo:hi]
        bank_lo = np.asarray(bank_bounds, dtype=np.int64)[(ks % N_BANKS)]
        bloc = (s_o[lo:hi] - bank_lo).astype(np.int16)
        idx16[pv % 16, chv * 8 + pv // 16] = bloc
        idx128 = np.tile(idx16, (8, 1))

        # tile-contiguous transposed x: row (t*128+p) holds feature-chunk p of
        # all 5 feature blocks for the 128 nodes of tile t
        xTt = np.zeros((N_TILES * P, 5 * P), dtype=BF16)
        xsh = x[c * NT : (c + 1) * NT, :].astype(BF16)   # [NT, 602]
        xp = np.zeros((N_TILES * P, F_PAD), dtype=BF16)
        xp[:NT, :F_IN] = xsh
        # xp: [t*128+n, c*128+p] -> xTt[t*128+p, c*128+n]
        x4 = xp.reshape(N_TILES, P, 5, P)                # [t, n, c, p]
        xTt = np.ascontiguousarray(
            x4.transpose(0, 3, 2, 1).reshape(N_TILES * P, 5 * P)
        )
        W1p = np.zeros((F_PAD, HID), dtype=BF16)
        W1p[:F_IN, :] = W1.astype(BF16)
        W2p = np.zeros((HID, HID2), dtype=BF16)
        W2p[:, :C] = W2.astype(BF16)
        iota = np.tile(np.arange(P, dtype=BF16), (P, 1))
        own = dinv[c * NT : (c + 1) * NT] ** 2
        dv2 = np.zeros((P, N_TILES), dtype=np.float32)
        for t in range(N_TILES):
            tsz = min(P, NT - t * P)
            dv2[:tsz, t] = own[t * P : t * P + tsz]
        in_maps.append(
            dict(
                xT=xTt,
                dinv2=dv2,
                ident=np.eye(P, dtype=BF16),
                W1p=W1p,
                b1=b1.reshape(HID, 1).astype(np.float32),
                W2p=W2p,
                b2bc=np.tile(b2.reshape(1, C), (P, 1)).astype(np.float32),
                iota=iota,
                idx16=idx128,
                erel=erel,
                enrm=enrm,
            )
        )
    plan = dict(NCH=NCH, cols=cols, ch_index=ch_index, groups=groups,
                group_start=group_start, call_plan=call_plan,
                bank_bounds=bank_bounds)
    return in_maps, plan


def _build(plan):
    NCH = plan["NCH"]
    cols = plan["cols"]
    ch_index = plan["ch_index"]
    groups = plan["groups"]
    group_start = plan["group_start"]
    call_plan = plan["call_plan"]
    bank_bounds = plan["bank_bounds"]
    f32 = mybir.dt.float32
    bf16 = mybir.dt.bfloat16

    nc = bacc.Bacc("TRN2", target_bir_lowering=False, num_devices=N_CORES,
                   num_swdge_queues=N_QUEUES)
    xT = nc.declare_dram_parameter("xT", [N_TILES * P, 5 * P], bf16, isOutput=False)
    W1p = nc.declare_dram_parameter("W1p", [F_PAD, HID], bf16, isOutput=False)
    b1p = nc.declare_dram_parameter("b1", [HID, 1], f32, isOutput=False)
    W2p = nc.declare_dram_parameter("W2p", [HID, HID2], bf16, isOutput=False)
    b2bc = nc.declare_dram_parameter("b2bc", [P, C], f32, isOutput=False)
    iota_in = nc.declare_dram_parameter("iota", [P, P], bf16, isOutput=False)
    ident_in = nc.declare_dram_parameter("ident", [P, P], bf16, isOutput=False)
    dinv2_in = nc.declare_dram_parameter("dinv2", [P, N_TILES], f32, isOutput=False)
    idx16_in = nc.declare_dram_parameter("idx16", [P, NCH * 8], mybir.dt.int16, isOutput=False)
    erel_in = nc.declare_dram_parameter("erel", [P, NCH], f32, isOutput=False)
    enrm_in = nc.declare_dram_parameter("enrm", [P, NCH], f32, isOutput=False)
    out_p = nc.declare_dram_parameter("out", [NT, C], f32, isOutput=True)

    h1_local = nc.dram_tensor("h1_local", [NT, HID2], bf16)
    h1_full = nc.dram_tensor("h1_full", [N, HID2], bf16, addr_space="Shared")
    h2_local = nc.dram_tensor("h2_local", [NT, HID2], bf16)
    h2_full = nc.dram_tensor("h2_full", [N, HID2], bf16, addr_space="Shared")

    rg = [list(range(N_CORES))]
    max_cols_g = max(
        int(cols[np.array(g), :].sum()) for g in groups
    )
    queue_of = {}
    qq = 0
    for (gg, b, ch0, cw) in call_plan:
        queue_of[(gg, b)] = qq % N_QUEUES
        qq += 1

    with tile.TileContext(nc) as tc, ExitStack() as ctx:
        consts = ctx.enter_context(tc.tile_pool(name="consts", bufs=1))
        xpool = ctx.enter_context(tc.tile_pool(name="xpool", bufs=3))
        psA = ctx.enter_context(tc.tile_pool(name="psA", bufs=2, space="PSUM"))
        big = ctx.enter_context(tc.tile_pool(name="big", bufs=1))
        gpool = ctx.enter_context(tc.tile_pool(name="gpool", bufs=2))
        spool = ctx.enter_context(tc.tile_pool(name="spool", bufs=3))
        ps1 = ctx.enter_context(tc.tile_pool(name="ps1", bufs=2, space="PSUM"))

        # ---- constants ----
        iota_t = consts.tile([P, P], bf16)
        nc.sync.dma_start(out=iota_t[:], in_=iota_in[:])
        ident_t = consts.tile([P, P], bf16)
        nc.sync.dma_start(out=ident_t[:], in_=ident_in[:])
        dinv2_t = consts.tile([P, N_TILES], f32)
        nc.sync.dma_start(out=dinv2_t[:], in_=dinv2_in[:])
        W1t = consts.tile([P, 5 * HID], bf16)
        W1t3 = W1t[:].rearrange("p (c h) -> p c h", c=5)
        nc.sync.dma_start(
            out=W1t3, in_=W1p[:].rearrange("(c p) h -> p c h", c=5)
        )
        b1t = consts.tile([HID, 1], f32)
        nc.sync.dma_start(out=b1t[:], in_=b1p[:])
        W2t = consts.tile([HID, HID2], bf16)
        nc.sync.dma_start(out=W2t[:], in_=W2p[:])
        b2t = consts.tile([P, C], f32)
        nc.sync.dma_start(out=b2t[:], in_=b2bc[:])
        idxt = consts.tile([P, NCH * 8], mybir.dt.int16)
        nc.sync.dma_start(out=idxt[:], in_=idx16_in[:])
        erelt = consts.tile([P, NCH], f32)
        nc.sync.dma_start(out=erelt[:], in_=erel_in[:])
        enrmt = consts.tile([P, NCH], f32)
        nc.sync.dma_start(out=enrmt[:], in_=enrm_in[:])

        # persistent per-layer SBUF copies of the local shard (self-loop rows)
        h1keep = big.tile([P, N_TILES * HID], bf16)
        h1k3 = h1keep[:].rearrange("p (t h) -> p t h", t=N_TILES)
        h2keep = big.tile([P, N_TILES * HID], bf16)
        h2k3 = h2keep[:].rearrange("p (t h) -> p t h", t=N_TILES)
        # the last tile is partial (84 rows): zero the never-written pad
        # partitions once so DD-masked matmuls multiply 0, not stale SBUF
        # (memset needs an aligned partition start; 64 covers rows 84..127,
        # the 64..83 overlap is overwritten by the later activation)
        nc.vector.memset(h1k3[64:, N_TILES - 1, :], 0.0)
        nc.vector.memset(h2k3[64:, N_TILES - 1, :], 0.0)

        # ---- phase 1: h1_local = x @ W1 ----
        for t in range(N_TILES):
            t0 = t * P
            tsz = min(P, NT - t0)
            xt = xpool.tile([P, 5 * P], bf16, tag="xt")
            xt3 = xt[:].rearrange("p (c n) -> p c n", c=5)
            nc.sync.dma_start(out=xt[:], in_=xT[t * P : (t + 1) * P, :])
            pa = psA.tile([P, HID], f32)
            for cb in range(5):
                nc.tensor.matmul(
                    out=pa[:tsz, :],
                    lhsT=xt3[:, cb, :tsz],
                    rhs=W1t3[:, cb, :],
                    start=(cb == 0),
                    stop=(cb == 4),
                )
            nc.scalar.activation(
                out=h1k3[:tsz, t, :], in_=pa[:tsz, :],
                func=mybir.ActivationFunctionType.Copy,
            )
            nc.sync.dma_start(
                out=h1_local[t0 : t0 + tsz, 0:HID], in_=h1k3[:tsz, t, :]
            )

        if SIM_MODE:
            nc.sync.dma_start(out=h1_full[0:NT, :], in_=h1_local[:])
        else:
            nc.gpsimd.collective_compute(
                "AllGather", mybir.AluOpType.bypass, replica_groups=rg,
                ins=[h1_local[:]], outs=[h1_full[:]],
            )

        # persistent relu(h1_agg)^T  [HID, NT]
        h1rT = big.tile([HID, NT], bf16)
        if VARIANT == "nocompute":
            nc.vector.memset(h1rT[:], 0.0)

        def conv(layer, h_full, hk3):
            """Aggregation sweep. layer=1: out h1rT (transposed, relu+b1).
            layer=2: +b2 into the batched logits buffer."""
            for gidx, g in enumerate(groups):
                gs = group_start[gidx]
                gout = gpool.tile([P, max_cols_g * HID2], bf16, tag="gout")
                g3 = gout[:].rearrange("p (c h) -> p c h", c=max_cols_g)
                for (gg, b, ch0, cw) in call_plan:
                    if gg != gidx:
                        continue
                    loc = ch0 - gs
                    if VARIANT == "nogather":
                        # keep the tile allocated without the gather traffic
                        nc.vector.memset(g3[:, loc : loc + cw, 0:2], 0.0)
                        continue
                    nc.gpsimd.dma_gather(
                        out_ap=g3[:, loc : loc + cw, :],
                        in_ap=h_full[bank_bounds[b] : bank_bounds[b + 1], :],
                        idxs_ap=idxt[:, ch0 * 8 : (ch0 + cw) * 8],
                        num_idxs=cw * P,
                        num_idxs_reg=cw * P,
                        elem_size=HID2,
                        single_packet=False,
                        queue_num=queue_of[(gg, b)],
                    )
                for t in g:
                    t0 = t * P
                    tsz = min(P, NT - t0)
                    n_ch_t = int(cols[t, :].sum()) + 1
                    if VARIANT == "nocompute":
                        continue
                    if layer == 1:
                        pt = ps1.tile([HID, P], f32, tag="ps_l1")
                    else:
                        pt = ps1.tile([P, HID], f32, tag="ps_l2")
                    # self-loop diagonal term: D = diag(dinv^2) over this tile,
                    # own rows come from the resident SBUF copy
                    DD = spool.tile([P, P], bf16, tag="DD")
                    nc.vector.tensor_scalar(
                        out=DD[:], in0=ident_t[:], scalar1=dinv2_t[:, t : t + 1],
                        scalar2=None, op0=mybir.AluOpType.mult,
                    )
                    if layer == 1:
                        nc.tensor.matmul(
                            out=pt[:], lhsT=hk3[:, t, :], rhs=DD[:],
                            start=True, stop=(n_ch_t == 1),
                        )
                    else:
                        nc.tensor.matmul(
                            out=pt[:], lhsT=DD[:], rhs=hk3[:, t, :],
                            start=True, stop=(n_ch_t == 1),
                        )
                    k = 1
                    for b in range(N_BANKS):
                        cw = int(cols[t, b])
                        if cw == 0:
                            continue
                        ch0 = int(ch_index[t, b])
                        loc = ch0 - gs
                        for j in range(cw):
                            ch = ch0 + j
                            # norm-scaled one-hot: S[p, n] = (iota==dst_rel[p]) * norm[p]
                            SS = spool.tile([P, P], bf16, tag="SS")
                            nc.vector.tensor_scalar(
                                out=SS[:], in0=iota_t[:],
                                scalar1=erelt[:, ch : ch + 1],
                                scalar2=enrmt[:, ch : ch + 1],
                                op0=mybir.AluOpType.is_equal,
                                op1=mybir.AluOpType.mult,
                            )
                            if layer == 1:
                                nc.tensor.matmul(
                                    out=pt[:],
                                    lhsT=g3[:, loc + j, 0:HID],
                                    rhs=SS[:],
                                    start=False,
                                    stop=(k == n_ch_t - 1),
                                )
                            else:
                                nc.tensor.matmul(
                                    out=pt[:],
                                    lhsT=SS[:],
                                    rhs=g3[:, loc + j, 0:HID],
                                    start=False,
                                    stop=(k == n_ch_t - 1),
                                )
                            k += 1
                    if layer == 1:
                        nc.scalar.activation(
                            out=h1rT[:, t0 : t0 + tsz], in_=pt[:, :tsz],
                            func=mybir.ActivationFunctionType.Relu,
                            bias=b1t[:],
                        )
                    else:
                        # L = psum + b2 into the batched logits buffer
                        nc.vector.tensor_tensor(
                            out=Lb3[:tsz, t, :], in0=pt[:tsz, :C],
                            in1=b2t[:tsz, :], op=mybir.AluOpType.add,
                        )

        conv(1, h1_full, h1k3)

        # ---- layer 2 linear: h2_local = relu(h1_agg) @ W2 (zero-padded cols) ----
        for t in range(N_TILES):
            t0 = t * P
            tsz = min(P, NT - t0)
            pb = psA.tile([P, HID2], f32, tag="ps_l2lin")
            nc.tensor.matmul(
                out=pb[:tsz, :], lhsT=h1rT[:, t0 : t0 + tsz], rhs=W2t[:],
                start=True, stop=True,
            )
            nc.scalar.activation(
                out=h2k3[:tsz, t, :], in_=pb[:tsz, 0:HID],
                func=mybir.ActivationFunctionType.Copy,
            )
            nc.sync.dma_start(
                out=h2_local[t0 : t0 + tsz, 0:HID], in_=h2k3[:tsz, t, :]
            )

        if SIM_MODE:
            nc.sync.dma_start(out=h2_full[0:NT, :], in_=h2_local[:])
        else:
            nc.gpsimd.collective_compute(
                "AllGather", mybir.AluOpType.bypass, replica_groups=rg,
                ins=[h2_local[:]], outs=[h2_full[:]],
            )

        Lbig = big.tile([P, N_TILES * C], f32)
        Lb3 = Lbig[:].rearrange("p (t c) -> p t c", t=N_TILES)
        if VARIANT == "nocompute":
            nc.vector.memset(Lbig[:], 0.0)

        conv(2, h2_full, h2k3)

        # ---- batched log_softmax over all tiles ----
        negm = big.tile([P, N_TILES], f32)
        nc.vector.tensor_reduce(
            out=negm[:], in_=Lb3, axis=mybir.AxisListType.X,
            op=mybir.AluOpType.max, negate=True,
        )
        # Lc = L - max (3D broadcast of negm), in place
        Lc = Lbig
        Lc3 = Lb3
        nc.vector.tensor_tensor(
            out=Lc3, in0=Lb3, in1=negm[:].to_broadcast([P, N_TILES, C]),
            op=mybir.AluOpType.add,
        )
        Eb = big.tile([P, N_TILES * C], f32)
        nc.scalar.activation(
            out=Eb[:], in_=Lc[:], func=mybir.ActivationFunctionType.Exp,
        )
        sums = big.tile([P, N_TILES], f32)
        nc.vector.tensor_reduce(
            out=sums[:], in_=Eb[:].rearrange("p (t c) -> p t c", t=N_TILES),
            axis=mybir.AxisListType.X, op=mybir.AluOpType.add,
        )
        lns = big.tile([P, N_TILES], f32)
        nc.scalar.activation(
            out=lns[:], in_=sums[:], func=mybir.ActivationFunctionType.Ln,
        )
        # out = Lc - ln(sum)
        nc.vector.tensor_tensor(
            out=Lc3, in0=Lc3, in1=lns[:].to_broadcast([P, N_TILES, C]),
            op=mybir.AluOpType.subtract,
        )
        # two DMAs: full tiles then the 84-row tail (rows beyond NT are garbage)
        nc.sync.dma_start(
            out=out_p[0 : (N_TILES - 1) * P, :].rearrange("(t p) c -> p t c", t=N_TILES - 1),
            in_=Lc3[:, : N_TILES - 1, :],
        )
        last0 = (N_TILES - 1) * P
        nc.sync.dma_start(
            out=out_p[last0:NT, :], in_=Lc3[: NT - last0, N_TILES - 1, :],
        )

    nc.compile()
    return nc


def kernel(x, src, dst, W1, b1, W2, b2):
    in_maps, plan = _preprocess(x, src, dst, W1, b1, W2, b2)
    nc = _build(plan)
    res = run_bass_kernel_spmd(
        nc, in_maps, list(range(N_CORES)), trace=PROFILE
    )
    _LAST_RESULTS["exec_time_ns"] = getattr(res, "exec_time_ns", None)
    _LAST_RESULTS["profile_json"] = getattr(res, "profile_json", None)
    out = np.concatenate([res.results[c]["out"] for c in range(N_CORES)], axis=0)
    return out.astype(np.float32)


# revision 19
# speedup vs baseline: 1.9821x; 1.9821x over previous
"""Trainium2 Bass kernel for 2-layer GCN forward (Reddit-like), 8-way node-sharded.

Strategy (matches the sharding hint):
- Nodes partitioned contiguously across 8 cores (12500 each). Edges (with
  self-loops) are routed to the core owning their destination node; within a
  core they are grouped by 128-node destination tile and by source "bank"
  (gather indices are int16, so the feature table is split into 4 banks).
- Per core: h1 = x_shard @ W1 on PE (bf16); the 12.5k-row shard is AllGathered
  in two halves (A: tiles 0-48, B: tiles 49-97) so the second half's collective
  overlaps the first half's gathers. The gather table layout is
  [all cores' A-halves | all cores' B-halves] so each half is one contiguous
  AllGather output; banks 0-1 live in region A, banks 2-3 in region B.
- Per edge-chunk of 128 edges, dma_gather the source rows (4 SWDGE queues),
  and segment-sum via a norm-scaled one-hot selection matrix matmul
  accumulated in PSUM (S[e, n] = (dst_rel[e] == n) * norm[e]).
  relu(+b1), @W2, AllGather (again split), same aggregation, +b2, log_softmax.
- All tensors feeding the PE are bf16 (1 cycle/row vs 4 for fp32); PSUM
  accumulation stays fp32. h tables are padded to 128 cols so gather rows are
  256B (DMA gather requires elem_size and row stride to be multiples of 256B).
- Self-loop rows are kept resident in SBUF (h1keep/h2keep).
- Weight matrices are replicated; all index preprocessing is host-side numpy.
"""
import math
import numpy as np
import ml_dtypes
from contextlib import ExitStack

import concourse.bass as bass
import concourse.bacc as bacc
import concourse.tile as tile
from concourse import mybir
from concourse.bass_utils import run_bass_kernel_spmd

BF16 = ml_dtypes.bfloat16

# problem sizes (hardcoded per the contract)
N = 100000
E = 1250000
F_IN = 602
F_PAD = 640          # 5 x 128
HID = 64
HID2 = 128           # padded h-table width (256B bf16 rows)
C = 41
N_CORES = 8
NT = N // N_CORES    # 12500 nodes per core
P = 128
N_TILES = (NT + P - 1) // P          # 98 (last tile 84 nodes)
TILES_A = 49                         # collective split: tiles 0..48 -> half A
NA = TILES_A * P                     # 6272 rows per core in half A
NB = NT - NA                         # 6228 rows per core in half B
RA = N_CORES * NA                    # 50176 table rows in region A
RB = N_CORES * NB                    # 49824 table rows in region B
N_BANKS = 4
GROUP = 6                            # node tiles per gather call group

PROFILE = False      # set True from test harness to request an NTFF trace
SIM_MODE = False     # replace collectives with local copies (TimelineSim only)
VARIANT = "full"     # timing decomposition: "full" | "nogather" | "nocompute"
N_QUEUES = 4         # SWDGE queues to spread gather calls across (1..4)
_LAST_RESULTS = {}   # debug: profile info stash


def _tune_banks(rowid, core_tile):
    """Hill-climb the intra-region bank boundaries (bank 0|1 split inside
    region A, bank 2|3 split inside region B) to minimize total chunk count
    sum over (tile,bank) of ceil(max_core_count/128)."""
    def nch(bA, bB):
        bounds = np.asarray([bA, RA, bB], dtype=np.int64)
        bank = np.searchsorted(bounds, rowid, side="right")
        key = core_tile * N_BANKS + bank
        counts = np.bincount(key, minlength=N_CORES * N_TILES * N_BANKS)
        counts = counts.reshape(N_CORES, N_TILES, N_BANKS)
        return int(np.ceil(counts.max(axis=0) / P).sum())

    bA, bB = RA // 2, RA + RB // 2
    best_v = nch(bA, bB)
    for step in (4000, 2000, 1000, 500, 250):
        improved = True
        while improved:
            improved = False
            for which in (0, 1):
                for sgn in (+1, -1):
                    cA, cB = bA, bB
                    if which == 0:
                        cA = bA + sgn * step
                        if not (RA - 32767 <= cA <= 32767):
                            continue
                    else:
                        cB = bB + sgn * step
                        if not (N - 32767 <= cB <= RA + 32767):
                            continue
                    v = nch(cA, cB)
                    if v < best_v:
                        bA, bB, best_v = cA, cB, v
                        improved = True
    return [0, bA, RA, bB, N], best_v


def _preprocess(x, src, dst, W1, b1, W2, b2):
    """Host-side index preprocessing and sharding. Returns in_maps + plan."""
    src = np.asarray(src).astype(np.int64).ravel()
    dst = np.asarray(dst).astype(np.int64).ravel()
    x = np.asarray(x, dtype=np.float32)
    W1 = np.asarray(W1, dtype=np.float32)
    b1 = np.asarray(b1, dtype=np.float32)
    W2 = np.asarray(W2, dtype=np.float32)
    b2 = np.asarray(b2, dtype=np.float32)

    # degrees include self-loops (reference semantics); the self-loop term
    # itself is applied on-device as a diagonal matmul, not via gathers.
    deg = (np.bincount(dst, minlength=N) + 1.0).astype(np.float32)
    dinv = (1.0 / np.sqrt(deg)).astype(np.float32)
    s = src
    d = dst
    norm = (dinv[s] * dinv[d]).astype(np.float32)

    core = d // NT
    trel = (d % NT) // P
    core_tile = core * N_TILES + trel

    # gather-table row of each source node: region A holds every core's first
    # NA rows, region B the rest
    sc = s // NT
    sr = s % NT
    rowid = np.where(sr < NA, sc * NA + sr, RA + sc * NB + (sr - NA)).astype(np.int64)

    bank_bounds, _ = _tune_banks(rowid, core_tile)
    bounds = np.asarray([bank_bounds[1], bank_bounds[2], bank_bounds[3]], dtype=np.int64)

    bank = np.searchsorted(bounds, rowid, side="right")
    key = (core_tile * N_BANKS + bank).astype(np.int64)
    order = np.argsort(key, kind="stable")
    row_o = rowid[order]
    d_o = d[order]
    n_o = norm[order]
    key_o = key[order]

    n_keys = N_CORES * N_TILES * N_BANKS
    counts = np.bincount(key_o, minlength=n_keys).reshape(N_CORES, N_TILES, N_BANKS)
    # uniform chunk counts across cores (single SPMD program)
    cols = np.ceil(counts.max(axis=0) / P).astype(np.int64)      # [N_TILES, N_BANKS]
    NCH = int(cols.sum())                                         # chunks per core

    # group boundaries for gather calls: groups of GROUP node tiles
    groups = [
        list(range(g, min(g + GROUP, N_TILES))) for g in range(0, N_TILES, GROUP)
    ]

    # re-order chunks canonically: for g: for b: for t in g: cols[t][b] chunks
    # chunk_index[t][b] = start in canonical order
    ch_index = np.zeros((N_TILES, N_BANKS), dtype=np.int64)
    call_plan = []  # per group: list of (b, ch0, n_chunks) with ch0 canonical
    group_start = []
    cursor = 0
    for g in groups:
        group_start.append(cursor)
        for b in range(N_BANKS):
            cw = int(cols[np.array(g), b].sum())
            if cw:
                call_plan.append((len(group_start) - 1, b, cursor, cw))
            for t in g:
                ch_index[t, b] = cursor
                cursor += int(cols[t, b])
    assert cursor == NCH

    # per-core edge placement
    in_maps = []
    for c in range(N_CORES):
        lo = np.searchsorted(key_o, c * N_TILES * N_BANKS)
        hi = np.searchsorted(key_o, (c + 1) * N_TILES * N_BANKS)
        ks = key_o[lo:hi] - c * N_TILES * N_BANKS       # (t * N_BANKS + b)
        # position within the (t,b) run for each edge
        run_starts = np.concatenate(
            [[0], np.cumsum(np.bincount(ks, minlength=N_TILES * N_BANKS))]
        )[:-1]
        i_local = np.arange(hi - lo) - run_starts[ks]
        chv = ch_index.ravel()[ks] + i_local // P
        pv = (i_local % P).astype(np.int64)

        erel = np.zeros((P, NCH), dtype=np.float32)
        enrm = np.zeros((P, NCH), dtype=np.float32)
        idx16 = np.zeros((16, NCH * 8), dtype=np.int16)
        drel = (d_o[lo:hi] - (c * NT + (ks // N_BANKS) * P)).astype(np.float32)
        erel[pv, chv] = drel
        enrm[pv, chv] = n_o[lo:hi]
        bank_lo = np.asarray(bank_bounds, dtype=np.int64)[(ks % N_BANKS)]
        bloc = (row_o[lo:hi] - bank_lo).astype(np.int16)
        idx16[pv % 16, chv * 8 + pv // 16] = bloc
        idx128 = np.tile(idx16, (8, 1))

        # tile-contiguous transposed x: row (t*128+p) holds feature-chunk p of
        # all 5 feature blocks for the 128 nodes of tile t
        xsh = x[c * NT : (c + 1) * NT, :].astype(BF16)   # [NT, 602]
        xp = np.zeros((N_TILES * P, F_PAD), dtype=BF16)
        xp[:NT, :F_IN] = xsh
        x4 = xp.reshape(N_TILES, P, 5, P)                # [t, n, c, p]
        xTt = np.ascontiguousarray(
            x4.transpose(0, 3, 2, 1).reshape(N_TILES * P, 5 * P)
        )
        W1p = np.zeros((F_PAD, HID), dtype=BF16)
        W1p[:F_IN, :] = W1.astype(BF16)
        W2p = np.zeros((HID, HID2), dtype=BF16)
        W2p[:, :C] = W2.astype(BF16)
        iota = np.tile(np.arange(P, dtype=BF16), (P, 1))
        own = dinv[c * NT : (c + 1) * NT] ** 2
        dv2 = np.zeros((P, N_TILES), dtype=np.float32)
        for t in range(N_TILES):
            tsz = min(P, NT - t * P)
            dv2[:tsz, t] = own[t * P : t * P + tsz]
        in_maps.append(
            dict(
                xT=xTt,
                dinv2=dv2,
                ident=np.eye(P, dtype=BF16),
                W1p=W1p,
                b1=b1.reshape(HID, 1).astype(np.float32),
                W2p=W2p,
                b2bc=np.tile(b2.reshape(1, C), (P, 1)).astype(np.float32),
                iota=iota,
                idx16=idx128,
                erel=erel,
                enrm=enrm,
            )
        )
    plan = dict(NCH=NCH, cols=cols, ch_index=ch_index, groups=groups,
                group_start=group_start, call_plan=call_plan,
                bank_bounds=bank_bounds)
    return in_maps, plan


def _build(plan):
    NCH = plan["NCH"]
    cols = plan["cols"]
    ch_index = plan["ch_index"]
    groups = plan["groups"]
    group_start = plan["group_start"]
    call_plan = plan["call_plan"]
    bank_bounds = plan["bank_bounds"]
    f32 = mybir.dt.float32
    bf16 = mybir.dt.bfloat16

    nc = bacc.Bacc("TRN2", target_bir_lowering=False, num_devices=N_CORES,
                   num_swdge_queues=N_QUEUES)
    xT = nc.declare_dram_parameter("xT", [N_TILES * P, 5 * P], bf16, isOutput=False)
    W1p = nc.declare_dram_parameter("W1p", [F_PAD, HID], bf16, isOutput=False)
    b1p = nc.declare_dram_parameter("b1", [HID, 1], f32, isOutput=False)
    W2p = nc.declare_dram_parameter("W2p", [HID, HID2], bf16, isOutput=False)
    b2bc = nc.declare_dram_parameter("b2bc", [P, C], f32, isOutput=False)
    iota_in = nc.declare_dram_parameter("iota", [P, P], bf16, isOutput=False)
    ident_in = nc.declare_dram_parameter("ident", [P, P], bf16, isOutput=False)
    dinv2_in = nc.declare_dram_parameter("dinv2", [P, N_TILES], f32, isOutput=False)
    idx16_in = nc.declare_dram_parameter("idx16", [P, NCH * 8], mybir.dt.int16, isOutput=False)
    erel_in = nc.declare_dram_parameter("erel", [P, NCH], f32, isOutput=False)
    enrm_in = nc.declare_dram_parameter("enrm", [P, NCH], f32, isOutput=False)
    out_p = nc.declare_dram_parameter("out", [NT, C], f32, isOutput=True)

    hloc = [
        (nc.dram_tensor(f"h{l}_localA", [NA, HID2], bf16),
         nc.dram_tensor(f"h{l}_localB", [NB, HID2], bf16))
        for l in (1, 2)
    ]
    hfull = [
        (nc.dram_tensor(f"h{l}_fullA", [RA, HID2], bf16, addr_space="Shared"),
         nc.dram_tensor(f"h{l}_fullB", [RB, HID2], bf16, addr_space="Shared"))
        for l in (1, 2)
    ]

    rg = [list(range(N_CORES))]
    max_cols_g = max(
        int(cols[np.array(g), :].sum()) for g in groups
    )
    queue_of = {}
    qq = 0
    for (gg, b, ch0, cw) in call_plan:
        queue_of[(gg, b)] = qq % N_QUEUES
        qq += 1

    def allgather(l, half):
        src = hloc[l][half]
        dstt = hfull[l][half]
        if SIM_MODE:
            nc.sync.dma_start(out=dstt[0 : (NA if half == 0 else NB), :], in_=src[:])
        else:
            nc.gpsimd.collective_compute(
                "AllGather", mybir.AluOpType.bypass, replica_groups=rg,
                ins=[src[:]], outs=[dstt[:]],
            )

    with tile.TileContext(nc) as tc, ExitStack() as ctx:
        consts = ctx.enter_context(tc.tile_pool(name="consts", bufs=1))
        xpool = ctx.enter_context(tc.tile_pool(name="xpool", bufs=3))
        psA = ctx.enter_context(tc.tile_pool(name="psA", bufs=2, space="PSUM"))
        big = ctx.enter_context(tc.tile_pool(name="big", bufs=1))
        gpool = ctx.enter_context(tc.tile_pool(name="gpool", bufs=3))
        spool = ctx.enter_context(tc.tile_pool(name="spool", bufs=6))
        ps1 = ctx.enter_context(tc.tile_pool(name="ps1", bufs=6, space="PSUM"))

        # ---- constants ----
        iota_t = consts.tile([P, P], bf16)
        nc.sync.dma_start(out=iota_t[:], in_=iota_in[:])
        ident_t = consts.tile([P, P], bf16)
        nc.sync.dma_start(out=ident_t[:], in_=ident_in[:])
        dinv2_t = consts.tile([P, N_TILES], f32)
        nc.sync.dma_start(out=dinv2_t[:], in_=dinv2_in[:])
        W1t = consts.tile([P, 5 * HID], bf16)
        W1t3 = W1t[:].rearrange("p (c h) -> p c h", c=5)
        nc.sync.dma_start(
            out=W1t3, in_=W1p[:].rearrange("(c p) h -> p c h", c=5)
        )
        b1t = consts.tile([HID, 1], f32)
        nc.sync.dma_start(out=b1t[:], in_=b1p[:])
        W2t = consts.tile([HID, HID2], bf16)
        nc.sync.dma_start(out=W2t[:], in_=W2p[:])
        b2t = consts.tile([P, C], f32)
        nc.sync.dma_start(out=b2t[:], in_=b2bc[:])
        idxt = consts.tile([P, NCH * 8], mybir.dt.int16)
        nc.sync.dma_start(out=idxt[:], in_=idx16_in[:])
        erelt = consts.tile([P, NCH], f32)
        nc.sync.dma_start(out=erelt[:], in_=erel_in[:])
        enrmt = consts.tile([P, NCH], f32)
        nc.sync.dma_start(out=enrmt[:], in_=enrm_in[:])

        # persistent per-layer SBUF copies of the local shard (self-loop rows)
        h1keep = big.tile([P, N_TILES * HID], bf16)
        h1k3 = h1keep[:].rearrange("p (t h) -> p t h", t=N_TILES)
        h2keep = big.tile([P, N_TILES * HID], bf16)
        h2k3 = h2keep[:].rearrange("p (t h) -> p t h", t=N_TILES)
        # the last tile is partial (84 rows): zero the never-written pad
        # partitions once so DD-masked matmuls multiply 0, not stale SBUF
        # (memset needs an aligned partition start; 64 covers rows 84..127,
        # the 64..83 overlap is overwritten by the later activation)
        nc.vector.memset(h1k3[64:, N_TILES - 1, :], 0.0)
        nc.vector.memset(h2k3[64:, N_TILES - 1, :], 0.0)

        # ---- phase 1: h1_local = x @ W1, AllGather in two halves ----
        for t in range(N_TILES):
            t0 = t * P
            tsz = min(P, NT - t0)
            xt = xpool.tile([P, 5 * P], bf16, tag="xt")
            xt3 = xt[:].rearrange("p (c n) -> p c n", c=5)
            nc.sync.dma_start(out=xt[:], in_=xT[t * P : (t + 1) * P, :])
            pa = psA.tile([P, HID2], f32, tag="psA")
            for cb in range(5):
                nc.tensor.matmul(
                    out=pa[:tsz, 0:HID],
                    lhsT=xt3[:, cb, :tsz],
                    rhs=W1t3[:, cb, :],
                    start=(cb == 0),
                    stop=(cb == 4),
                )
            nc.scalar.activation(
                out=h1k3[:tsz, t, :], in_=pa[:tsz, 0:HID],
                func=mybir.ActivationFunctionType.Copy,
            )
            if t < TILES_A:
                nc.sync.dma_start(
                    out=hloc[0][0][t0 : t0 + tsz, 0:HID], in_=h1k3[:tsz, t, :]
                )
            else:
                nc.sync.dma_start(
                    out=hloc[0][1][t0 - NA : t0 - NA + tsz, 0:HID],
                    in_=h1k3[:tsz, t, :],
                )
            if t == TILES_A - 1:
                allgather(0, 0)
        allgather(0, 1)

        # persistent relu(h1_agg)^T  [HID, NT]
        h1rT = big.tile([HID, NT], bf16)
        if VARIANT == "nocompute":
            nc.vector.memset(h1rT[:], 0.0)

        def gather_in_ap(l, b):
            half = 0 if b < 2 else 1
            base = 0 if half == 0 else RA
            return hfull[l][half][bank_bounds[b] - base : bank_bounds[b + 1] - base, :]

        def conv(layer, hk3):
            """Aggregation sweep. layer=1: out h1rT (transposed, relu+b1).
            layer=2: +b2 into the batched logits buffer."""
            li = layer - 1
            for gidx, g in enumerate(groups):
                gs = group_start[gidx]
                gout = gpool.tile([P, max_cols_g * HID2], bf16, tag="gout")
                g3 = gout[:].rearrange("p (c h) -> p c h", c=max_cols_g)
                for (gg, b, ch0, cw) in call_plan:
                    if gg != gidx:
                        continue
                    loc = ch0 - gs
                    if VARIANT == "nogather":
                        # keep the tile allocated without the gather traffic
                        nc.vector.memset(g3[:, loc : loc + cw, 0:2], 0.0)
                        continue
                    nc.gpsimd.dma_gather(
                        out_ap=g3[:, loc : loc + cw, :],
                        in_ap=gather_in_ap(li, b),
                        idxs_ap=idxt[:, ch0 * 8 : (ch0 + cw) * 8],
                        num_idxs=cw * P,
                        num_idxs_reg=cw * P,
                        elem_size=HID2,
                        single_packet=False,
                        queue_num=queue_of[(gg, b)],
                    )
                for t in g:
                    t0 = t * P
                    tsz = min(P, NT - t0)
                    n_ch_t = int(cols[t, :].sum()) + 1
                    if VARIANT == "nocompute":
                        continue
                    ptf = ps1.tile([P, P], f32, tag="ps_conv")
                    if layer == 1:
                        pt = ptf[0:HID, :]
                    else:
                        pt = ptf[:, 0:HID]
                    # self-loop diagonal term: D = diag(dinv^2) over this tile,
                    # own rows come from the resident SBUF copy
                    DD = spool.tile([P, P], bf16, tag="DD")
                    nc.vector.tensor_scalar(
                        out=DD[:], in0=ident_t[:], scalar1=dinv2_t[:, t : t + 1],
                        scalar2=None, op0=mybir.AluOpType.mult,
                    )
                    if layer == 1:
                        nc.tensor.matmul(
                            out=pt[:], lhsT=hk3[:, t, :], rhs=DD[:],
                            start=True, stop=(n_ch_t == 1),
                        )
                    else:
                        nc.tensor.matmul(
                            out=pt[:], lhsT=DD[:], rhs=hk3[:, t, :],
                            start=True, stop=(n_ch_t == 1),
                        )
                    k = 1
                    for b in range(N_BANKS):
                        cw = int(cols[t, b])
                        if cw == 0:
                            continue
                        ch0 = int(ch_index[t, b])
                        loc = ch0 - gs
                        for j in range(cw):
                            ch = ch0 + j
                            # norm-scaled one-hot: S[p, n] = (iota==dst_rel[p]) * norm[p]
                            SS = spool.tile([P, P], bf16, tag="SS")
                            nc.vector.tensor_scalar(
                                out=SS[:], in0=iota_t[:],
                                scalar1=erelt[:, ch : ch + 1],
                                scalar2=enrmt[:, ch : ch + 1],
                                op0=mybir.AluOpType.is_equal,
                                op1=mybir.AluOpType.mult,
                            )
                            if layer == 1:
                                nc.tensor.matmul(
                                    out=pt[:],
                                    lhsT=g3[:, loc + j, 0:HID],
                                    rhs=SS[:],
                                    start=False,
                                    stop=(k == n_ch_t - 1),
                                )
                            else:
                                nc.tensor.matmul(
                                    out=pt[:],
                                    lhsT=SS[:],
                                    rhs=g3[:, loc + j, 0:HID],
                                    start=False,
                                    stop=(k == n_ch_t - 1),
                                )
                            k += 1
                    if layer == 1:
                        nc.scalar.activation(
                            out=h1rT[:, t0 : t0 + tsz], in_=pt[:, :tsz],
                            func=mybir.ActivationFunctionType.Relu,
                            bias=b1t[:],
                        )
                    else:
                        # L = psum + b2 into the batched logits buffer
                        nc.vector.tensor_tensor(
                            out=Lb3[:tsz, t, :], in0=pt[:tsz, :C],
                            in1=b2t[:tsz, :], op=mybir.AluOpType.add,
                        )

        conv(1, h1k3)

        # ---- layer 2 linear: h2_local = relu(h1_agg) @ W2 (zero-padded cols),
        # AllGather in two halves ----
        for t in range(N_TILES):
            t0 = t * P
            tsz = min(P, NT - t0)
            pb = psA.tile([P, HID2], f32, tag="psA")
            nc.tensor.matmul(
                out=pb[:tsz, :], lhsT=h1rT[:, t0 : t0 + tsz], rhs=W2t[:],
                start=True, stop=True,
            )
            nc.scalar.activation(
                out=h2k3[:tsz, t, :], in_=pb[:tsz, 0:HID],
                func=mybir.ActivationFunctionType.Copy,
            )
            if t < TILES_A:
                nc.sync.dma_start(
                    out=hloc[1][0][t0 : t0 + tsz, 0:HID], in_=h2k3[:tsz, t, :]
                )
            else:
                nc.sync.dma_start(
                    out=hloc[1][1][t0 - NA : t0 - NA + tsz, 0:HID],
                    in_=h2k3[:tsz, t, :],
                )
            if t == TILES_A - 1:
                allgather(1, 0)
        allgather(1, 1)

        Lbig = big.tile([P, N_TILES * C], f32)
        Lb3 = Lbig[:].rearrange("p (t c) -> p t c", t=N_TILES)
        if VARIANT == "nocompute":
            nc.vector.memset(Lbig[:], 0.0)

        conv(2, h2k3)

        # ---- batched log_softmax over all tiles ----
        negm = big.tile([P, N_TILES], f32)
        nc.vector.tensor_reduce(
            out=negm[:], in_=Lb3, axis=mybir.AxisListType.X,
            op=mybir.AluOpType.max, negate=True,
        )
        # Lc = L - max (3D broadcast of negm), in place
        Lc = Lbig
        Lc3 = Lb3
        nc.vector.tensor_tensor(
            out=Lc3, in0=Lb3, in1=negm[:].to_broadcast([P, N_TILES, C]),
            op=mybir.AluOpType.add,
        )
        Eb = big.tile([P, N_TILES * C], f32)
        nc.scalar.activation(
            out=Eb[:], in_=Lc[:], func=mybir.ActivationFunctionType.Exp,
        )
        sums = big.tile([P, N_TILES], f32)
        nc.vector.tensor_reduce(
            out=sums[:], in_=Eb[:].rearrange("p (t c) -> p t c", t=N_TILES),
            axis=mybir.AxisListType.X, op=mybir.AluOpType.add,
        )
        lns = big.tile([P, N_TILES], f32)
        nc.scalar.activation(
            out=lns[:], in_=sums[:], func=mybir.ActivationFunctionType.Ln,
        )
        # out = Lc - ln(sum)
        nc.vector.tensor_tensor(
            out=Lc3, in0=Lc3, in1=lns[:].to_broadcast([P, N_TILES, C]),
            op=mybir.AluOpType.subtract,
        )
        # two DMAs: full tiles then the 84-row tail (rows beyond NT are garbage)
        nc.sync.dma_start(
            out=out_p[0 : (N_TILES - 1) * P, :].rearrange("(t p) c -> p t c", t=N_TILES - 1),
            in_=Lc3[:, : N_TILES - 1, :],
        )
        last0 = (N_TILES - 1) * P
        nc.sync.dma_start(
            out=out_p[last0:NT, :], in_=Lc3[: NT - last0, N_TILES - 1, :],
        )

    nc.compile()
    return nc


def kernel(x, src, dst, W1, b1, W2, b2):
    in_maps, plan = _preprocess(x, src, dst, W1, b1, W2, b2)
    nc = _build(plan)
    res = run_bass_kernel_spmd(
        nc, in_maps, list(range(N_CORES)), trace=PROFILE
    )
    _LAST_RESULTS["exec_time_ns"] = getattr(res, "exec_time_ns", None)
    _LAST_RESULTS["profile_json"] = getattr(res, "profile_json", None)
    out = np.concatenate([res.results[c]["out"] for c in range(N_CORES)], axis=0)
    return out.astype(np.float32)


# revision 25
# speedup vs baseline: 4.8593x; 2.4515x over previous
"""Trainium2 Bass kernel for 2-layer GCN forward (Reddit-like), 8-way node-sharded.

Strategy (matches the sharding hint):
- Nodes partitioned contiguously across 8 cores (12500 each). Edges (with
  self-loops) are routed to the core owning their destination node; within a
  core they are grouped by 128-node destination tile and by source "bank"
  (gather indices are int16, so the feature table is split into 4 banks).
- Per core: h1 = x_shard @ W1 on PE (bf16); the 12.5k-row shard is AllGathered
  in two halves (A: tiles 0-48, B: tiles 49-97) so the second half's collective
  overlaps the first half's gathers. The gather table layout is
  [all cores' A-halves | all cores' B-halves] so each half is one contiguous
  AllGather output; banks 0-1 live in region A, banks 2-3 in region B.
- Per edge-chunk of 128 edges, dma_gather the source rows (4 SWDGE queues),
  and segment-sum via a norm-scaled one-hot selection matrix matmul
  accumulated in PSUM (S[e, n] = (dst_rel[e] == n) * norm[e]).
  relu(+b1), @W2, AllGather (again split), same aggregation, +b2, log_softmax.
- All tensors feeding the PE are bf16 (1 cycle/row vs 4 for fp32); PSUM
  accumulation stays fp32. h tables are padded to 128 cols so gather rows are
  256B (DMA gather requires elem_size and row stride to be multiples of 256B).
- Self-loop rows are kept resident in SBUF (h1keep/h2keep).
- Weight matrices are replicated; all index preprocessing is host-side numpy.
"""
import math
import numpy as np
import ml_dtypes
from contextlib import ExitStack

import concourse.bass as bass
import concourse.bacc as bacc
import concourse.tile as tile
from concourse import mybir
from concourse.bass_utils import run_bass_kernel_spmd

BF16 = ml_dtypes.bfloat16

# problem sizes (hardcoded per the contract)
N = 100000
E = 1250000
F_IN = 602
F_PAD = 640          # 5 x 128
HID = 64
HID2 = 128           # padded h-table width (256B bf16 rows)
C = 41
N_CORES = 8
NT = N // N_CORES    # 12500 nodes per core
P = 128
N_TILES = (NT + P - 1) // P          # 98 (last tile 84 nodes)
TILES_A = 49                         # collective split: tiles 0..48 -> half A
NA = TILES_A * P                     # 6272 rows per core in half A
NB = NT - NA                         # 6228 rows per core in half B
RA = N_CORES * NA                    # 50176 table rows in region A
RB = N_CORES * NB                    # 49824 table rows in region B
N_BANKS = 4
GROUP = 8                            # node tiles per gather call group

PROFILE = False      # set True from test harness to request an NTFF trace
SIM_MODE = False     # replace collectives with local copies (TimelineSim only)
VARIANT = "full"     # timing decomposition: "full" | "nogather" | "nocompute"
N_QUEUES = 4         # SWDGE queues to spread gather calls across (1..4)
SINGLE_PACKET = False  # dma_gather single_packet flag
SPLIT_MID = True     # issue half-A AllGather mid-phase (True) or both at end
REPEAT = 1           # run the whole body this many times (timing NEFFs)
PS1_BUFS = 6
SPOOL_BUFS = 6
GPOOL_BUFS = 2
_LAST_RESULTS = {}   # debug: profile info stash


def _tune_banks(rowid, core_tile):
    """Hill-climb the intra-region bank boundaries (bank 0|1 split inside
    region A, bank 2|3 split inside region B) to minimize total chunk count
    sum over (tile,bank) of ceil(max_core_count/128)."""
    def nch(bA, bB):
        bounds = np.asarray([bA, RA, bB], dtype=np.int64)
        bank = np.searchsorted(bounds, rowid, side="right")
        key = core_tile * N_BANKS + bank
        counts = np.bincount(key, minlength=N_CORES * N_TILES * N_BANKS)
        counts = counts.reshape(N_CORES, N_TILES, N_BANKS)
        return int(np.ceil(counts.max(axis=0) / P).sum())

    bA, bB = RA // 2, RA + RB // 2
    best_v = nch(bA, bB)
    for step in (4000, 2000, 1000, 500, 250):
        improved = True
        while improved:
            improved = False
            for which in (0, 1):
                for sgn in (+1, -1):
                    cA, cB = bA, bB
                    if which == 0:
                        cA = bA + sgn * step
                        if not (RA - 32767 <= cA <= 32767):
                            continue
                    else:
                        cB = bB + sgn * step
                        if not (N - 32767 <= cB <= RA + 32767):
                            continue
                    v = nch(cA, cB)
                    if v < best_v:
                        bA, bB, best_v = cA, cB, v
                        improved = True
    return [0, bA, RA, bB, N], best_v


def _preprocess(x, src, dst, W1, b1, W2, b2):
    """Host-side index preprocessing and sharding. Returns in_maps + plan."""
    src = np.asarray(src).astype(np.int64).ravel()
    dst = np.asarray(dst).astype(np.int64).ravel()
    x = np.asarray(x, dtype=np.float32)
    W1 = np.asarray(W1, dtype=np.float32)
    b1 = np.asarray(b1, dtype=np.float32)
    W2 = np.asarray(W2, dtype=np.float32)
    b2 = np.asarray(b2, dtype=np.float32)

    # degrees include self-loops (reference semantics); the self-loop term
    # itself is applied on-device as a diagonal matmul, not via gathers.
    deg = (np.bincount(dst, minlength=N) + 1.0).astype(np.float32)
    dinv = (1.0 / np.sqrt(deg)).astype(np.float32)
    s = src
    d = dst
    norm = (dinv[s] * dinv[d]).astype(np.float32)

    core = d // NT
    trel = (d % NT) // P
    core_tile = core * N_TILES + trel

    # gather-table row of each source node: region A holds every core's first
    # NA rows, region B the rest
    sc = s // NT
    sr = s % NT
    rowid = np.where(sr < NA, sc * NA + sr, RA + sc * NB + (sr - NA)).astype(np.int64)

    bank_bounds, _ = _tune_banks(rowid, core_tile)
    bounds = np.asarray([bank_bounds[1], bank_bounds[2], bank_bounds[3]], dtype=np.int64)

    bank = np.searchsorted(bounds, rowid, side="right")
    key = (core_tile * N_BANKS + bank).astype(np.int64)
    order = np.argsort(key, kind="stable")
    row_o = rowid[order]
    d_o = d[order]
    n_o = norm[order]
    key_o = key[order]

    n_keys = N_CORES * N_TILES * N_BANKS
    counts = np.bincount(key_o, minlength=n_keys).reshape(N_CORES, N_TILES, N_BANKS)
    # uniform chunk counts across cores (single SPMD program)
    cols = np.ceil(counts.max(axis=0) / P).astype(np.int64)      # [N_TILES, N_BANKS]
    NCH = int(cols.sum())                                         # chunks per core

    # group boundaries for gather calls: groups of GROUP node tiles
    groups = [
        list(range(g, min(g + GROUP, N_TILES))) for g in range(0, N_TILES, GROUP)
    ]

    # re-order chunks canonically: for g: for b: for t in g: cols[t][b] chunks
    # chunk_index[t][b] = start in canonical order
    ch_index = np.zeros((N_TILES, N_BANKS), dtype=np.int64)
    call_plan = []  # per group: list of (b, ch0, n_chunks) with ch0 canonical
    group_start = []
    cursor = 0
    for g in groups:
        group_start.append(cursor)
        for b in range(N_BANKS):
            cw = int(cols[np.array(g), b].sum())
            if cw:
                call_plan.append((len(group_start) - 1, b, cursor, cw))
            for t in g:
                ch_index[t, b] = cursor
                cursor += int(cols[t, b])
    assert cursor == NCH

    # per-core edge placement
    in_maps = []
    for c in range(N_CORES):
        lo = np.searchsorted(key_o, c * N_TILES * N_BANKS)
        hi = np.searchsorted(key_o, (c + 1) * N_TILES * N_BANKS)
        ks = key_o[lo:hi] - c * N_TILES * N_BANKS       # (t * N_BANKS + b)
        # position within the (t,b) run for each edge
        run_starts = np.concatenate(
            [[0], np.cumsum(np.bincount(ks, minlength=N_TILES * N_BANKS))]
        )[:-1]
        i_local = np.arange(hi - lo) - run_starts[ks]
        chv = ch_index.ravel()[ks] + i_local // P
        pv = (i_local % P).astype(np.int64)

        erel = np.zeros((P, NCH), dtype=np.float32)
        enrm = np.zeros((P, NCH), dtype=np.float32)
        idx16 = np.zeros((16, NCH * 8), dtype=np.int16)
        drel = (d_o[lo:hi] - (c * NT + (ks // N_BANKS) * P)).astype(np.float32)
        erel[pv, chv] = drel
        enrm[pv, chv] = n_o[lo:hi]
        bank_lo = np.asarray(bank_bounds, dtype=np.int64)[(ks % N_BANKS)]
        bloc = (row_o[lo:hi] - bank_lo).astype(np.int16)
        idx16[pv % 16, chv * 8 + pv // 16] = bloc
        idx128 = np.tile(idx16, (8, 1))

        # tile-contiguous transposed x: row (t*128+p) holds feature-chunk p of
        # all 5 feature blocks for the 128 nodes of tile t
        xsh = x[c * NT : (c + 1) * NT, :].astype(BF16)   # [NT, 602]
        xp = np.zeros((N_TILES * P, F_PAD), dtype=BF16)
        xp[:NT, :F_IN] = xsh
        x4 = xp.reshape(N_TILES, P, 5, P)                # [t, n, c, p]
        xTt = np.ascontiguousarray(
            x4.transpose(0, 3, 2, 1).reshape(N_TILES * P, 5 * P)
        )
        W1p = np.zeros((F_PAD, HID), dtype=BF16)
        W1p[:F_IN, :] = W1.astype(BF16)
        W2p = np.zeros((HID, HID2), dtype=BF16)
        W2p[:, :C] = W2.astype(BF16)
        iota = np.tile(np.arange(P, dtype=BF16), (P, 1))
        own = dinv[c * NT : (c + 1) * NT] ** 2
        dv2 = np.zeros((P, N_TILES), dtype=np.float32)
        for t in range(N_TILES):
            tsz = min(P, NT - t * P)
            dv2[:tsz, t] = own[t * P : t * P + tsz]
        in_maps.append(
            dict(
                xT=xTt,
                dinv2=dv2,
                ident=np.eye(P, dtype=BF16),
                W1p=W1p,
                b1=b1.reshape(HID, 1).astype(np.float32),
                W2p=W2p,
                b2bc=np.tile(b2.reshape(1, C), (P, 1)).astype(np.float32),
                iota=iota,
                idx16=idx128,
                erel=erel,
                enrm=enrm,
            )
        )
    plan = dict(NCH=NCH, cols=cols, ch_index=ch_index, groups=groups,
                group_start=group_start, call_plan=call_plan,
                bank_bounds=bank_bounds)
    return in_maps, plan


def _build(plan):
    NCH = plan["NCH"]
    cols = plan["cols"]
    ch_index = plan["ch_index"]
    groups = plan["groups"]
    group_start = plan["group_start"]
    call_plan = plan["call_plan"]
    bank_bounds = plan["bank_bounds"]
    f32 = mybir.dt.float32
    bf16 = mybir.dt.bfloat16

    nc = bacc.Bacc("TRN2", target_bir_lowering=False, num_devices=N_CORES,
                   num_swdge_queues=N_QUEUES)
    xT = nc.declare_dram_parameter("xT", [N_TILES * P, 5 * P], bf16, isOutput=False)
    W1p = nc.declare_dram_parameter("W1p", [F_PAD, HID], bf16, isOutput=False)
    b1p = nc.declare_dram_parameter("b1", [HID, 1], f32, isOutput=False)
    W2p = nc.declare_dram_parameter("W2p", [HID, HID2], bf16, isOutput=False)
    b2bc = nc.declare_dram_parameter("b2bc", [P, C], f32, isOutput=False)
    iota_in = nc.declare_dram_parameter("iota", [P, P], bf16, isOutput=False)
    ident_in = nc.declare_dram_parameter("ident", [P, P], bf16, isOutput=False)
    dinv2_in = nc.declare_dram_parameter("dinv2", [P, N_TILES], f32, isOutput=False)
    idx16_in = nc.declare_dram_parameter("idx16", [P, NCH * 8], mybir.dt.int16, isOutput=False)
    erel_in = nc.declare_dram_parameter("erel", [P, NCH], f32, isOutput=False)
    enrm_in = nc.declare_dram_parameter("enrm", [P, NCH], f32, isOutput=False)
    out_p = nc.declare_dram_parameter("out", [NT, C], f32, isOutput=True)

    hloc = [
        (nc.dram_tensor(f"h{l}_localA", [NA, HID2], bf16),
         nc.dram_tensor(f"h{l}_localB", [NB, HID2], bf16))
        for l in (1, 2)
    ]
    hfull = [
        (nc.dram_tensor(f"h{l}_fullA", [RA, HID2], bf16, addr_space="Shared"),
         nc.dram_tensor(f"h{l}_fullB", [RB, HID2], bf16, addr_space="Shared"))
        for l in (1, 2)
    ]

    rg = [list(range(N_CORES))]
    max_cols_g = max(
        int(cols[np.array(g), :].sum()) for g in groups
    )
    queue_of = {}
    qq = 0
    for (gg, b, ch0, cw) in call_plan:
        queue_of[(gg, b)] = qq % N_QUEUES
        qq += 1

    def allgather(l, half):
        src = hloc[l][half]
        dstt = hfull[l][half]
        if SIM_MODE:
            nc.sync.dma_start(out=dstt[0 : (NA if half == 0 else NB), :], in_=src[:])
        else:
            nc.gpsimd.collective_compute(
                "AllGather", mybir.AluOpType.bypass, replica_groups=rg,
                ins=[src[:]], outs=[dstt[:]],
            )

    with tile.TileContext(nc) as tc, ExitStack() as ctx:
        consts = ctx.enter_context(tc.tile_pool(name="consts", bufs=1))
        xpool = ctx.enter_context(tc.tile_pool(name="xpool", bufs=3))
        psA = ctx.enter_context(tc.tile_pool(name="psA", bufs=2, space="PSUM"))
        big = ctx.enter_context(tc.tile_pool(name="big", bufs=1))
        gpool = ctx.enter_context(tc.tile_pool(name="gpool", bufs=GPOOL_BUFS))
        spool = ctx.enter_context(tc.tile_pool(name="spool", bufs=SPOOL_BUFS))
        ps1 = ctx.enter_context(tc.tile_pool(name="ps1", bufs=PS1_BUFS, space="PSUM"))

        # ---- constants ----
        iota_t = consts.tile([P, P], bf16)
        nc.sync.dma_start(out=iota_t[:], in_=iota_in[:])
        ident_t = consts.tile([P, P], bf16)
        nc.sync.dma_start(out=ident_t[:], in_=ident_in[:])
        dinv2_t = consts.tile([P, N_TILES], f32)
        nc.sync.dma_start(out=dinv2_t[:], in_=dinv2_in[:])
        W1t = consts.tile([P, 5 * HID], bf16)
        W1t3 = W1t[:].rearrange("p (c h) -> p c h", c=5)
        nc.sync.dma_start(
            out=W1t3, in_=W1p[:].rearrange("(c p) h -> p c h", c=5)
        )
        b1t = consts.tile([HID, 1], f32)
        nc.sync.dma_start(out=b1t[:], in_=b1p[:])
        W2t = consts.tile([HID, HID2], bf16)
        nc.sync.dma_start(out=W2t[:], in_=W2p[:])
        b2t = consts.tile([P, C], f32)
        nc.sync.dma_start(out=b2t[:], in_=b2bc[:])
        idxt = consts.tile([P, NCH * 8], mybir.dt.int16)
        nc.sync.dma_start(out=idxt[:], in_=idx16_in[:])
        erelt = consts.tile([P, NCH], f32)
        nc.sync.dma_start(out=erelt[:], in_=erel_in[:])
        enrmt = consts.tile([P, NCH], f32)
        nc.sync.dma_start(out=enrmt[:], in_=enrm_in[:])

        # persistent per-layer SBUF copies of the local shard (self-loop rows)
        h1keep = big.tile([P, N_TILES * HID], bf16)
        h1k3 = h1keep[:].rearrange("p (t h) -> p t h", t=N_TILES)
        h2keep = big.tile([P, N_TILES * HID], bf16)
        h2k3 = h2keep[:].rearrange("p (t h) -> p t h", t=N_TILES)
        # the last tile is partial (84 rows): zero the never-written pad
        # partitions once so DD-masked matmuls multiply 0, not stale SBUF
        # (memset needs an aligned partition start; 64 covers rows 84..127,
        # the 64..83 overlap is overwritten by the later activation)
        nc.vector.memset(h1k3[64:, N_TILES - 1, :], 0.0)
        nc.vector.memset(h2k3[64:, N_TILES - 1, :], 0.0)

        # ---- phase 1: h1_local = x @ W1, AllGather in two halves ----
        for t in range(N_TILES):
            t0 = t * P
            tsz = min(P, NT - t0)
            xt = xpool.tile([P, 5 * P], bf16, tag="xt")
            xt3 = xt[:].rearrange("p (c n) -> p c n", c=5)
            nc.sync.dma_start(out=xt[:], in_=xT[t * P : (t + 1) * P, :])
            pa = psA.tile([P, HID2], f32, tag="psA")
            for cb in range(5):
                nc.tensor.matmul(
                    out=pa[:tsz, 0:HID],
                    lhsT=xt3[:, cb, :tsz],
                    rhs=W1t3[:, cb, :],
                    start=(cb == 0),
                    stop=(cb == 4),
                )
            nc.scalar.activation(
                out=h1k3[:tsz, t, :], in_=pa[:tsz, 0:HID],
                func=mybir.ActivationFunctionType.Copy,
            )
            if t < TILES_A:
                nc.sync.dma_start(
                    out=hloc[0][0][t0 : t0 + tsz, 0:HID], in_=h1k3[:tsz, t, :]
                )
            else:
                nc.sync.dma_start(
                    out=hloc[0][1][t0 - NA : t0 - NA + tsz, 0:HID],
                    in_=h1k3[:tsz, t, :],
                )
            if t == TILES_A - 1:
                allgather(0, 0)
        allgather(0, 1)

        # persistent relu(h1_agg)^T  [HID, NT]
        h1rT = big.tile([HID, NT], bf16)
        if VARIANT == "nocompute":
            nc.vector.memset(h1rT[:], 0.0)

        def gather_in_ap(l, b):
            half = 0 if b < 2 else 1
            base = 0 if half == 0 else RA
            return hfull[l][half][bank_bounds[b] - base : bank_bounds[b + 1] - base, :]

        def conv(layer, hk3):
            """Aggregation sweep. layer=1: out h1rT (transposed, relu+b1).
            layer=2: +b2 into the batched logits buffer."""
            li = layer - 1
            for gidx, g in enumerate(groups):
                gs = group_start[gidx]
                gout = gpool.tile([P, max_cols_g * HID2], bf16, tag="gout")
                g3 = gout[:].rearrange("p (c h) -> p c h", c=max_cols_g)
                for (gg, b, ch0, cw) in call_plan:
                    if gg != gidx:
                        continue
                    loc = ch0 - gs
                    if VARIANT == "nogather":
                        # keep the tile allocated without the gather traffic
                        nc.vector.memset(g3[:, loc : loc + cw, 0:2], 0.0)
                        continue
                    nc.gpsimd.dma_gather(
                        out_ap=g3[:, loc : loc + cw, :],
                        in_ap=gather_in_ap(li, b),
                        idxs_ap=idxt[:, ch0 * 8 : (ch0 + cw) * 8],
                        num_idxs=cw * P,
                        num_idxs_reg=cw * P,
                        elem_size=HID2,
                        single_packet=SINGLE_PACKET,
                        queue_num=queue_of[(gg, b)],
                    )
                for t in g:
                    t0 = t * P
                    tsz = min(P, NT - t0)
                    n_ch_t = int(cols[t, :].sum()) + 1
                    if VARIANT == "nocompute":
                        continue
                    ptf = ps1.tile([P, P], f32, tag="ps_conv")
                    if layer == 1:
                        pt = ptf[0:HID, :]
                    else:
                        pt = ptf[:, 0:HID]
                    # self-loop diagonal term: D = diag(dinv^2) over this tile,
                    # own rows come from the resident SBUF copy
                    DD = spool.tile([P, P], bf16, tag="DD")
                    nc.vector.tensor_scalar(
                        out=DD[:], in0=ident_t[:], scalar1=dinv2_t[:, t : t + 1],
                        scalar2=None, op0=mybir.AluOpType.mult,
                    )
                    if layer == 1:
                        nc.tensor.matmul(
                            out=pt[:], lhsT=hk3[:, t, :], rhs=DD[:],
                            start=True, stop=(n_ch_t == 1),
                        )
                    else:
                        nc.tensor.matmul(
                            out=pt[:], lhsT=DD[:], rhs=hk3[:, t, :],
                            start=True, stop=(n_ch_t == 1),
                        )
                    k = 1
                    for b in range(N_BANKS):
                        cw = int(cols[t, b])
                        if cw == 0:
                            continue
                        ch0 = int(ch_index[t, b])
                        loc = ch0 - gs
                        for j in range(cw):
                            ch = ch0 + j
                            # norm-scaled one-hot: S[p, n] = (iota==dst_rel[p]) * norm[p]
                            SS = spool.tile([P, P], bf16, tag="SS")
                            nc.vector.tensor_scalar(
                                out=SS[:], in0=iota_t[:],
                                scalar1=erelt[:, ch : ch + 1],
                                scalar2=enrmt[:, ch : ch + 1],
                                op0=mybir.AluOpType.is_equal,
                                op1=mybir.AluOpType.mult,
                            )
                            if layer == 1:
                                nc.tensor.matmul(
                                    out=pt[:],
                                    lhsT=g3[:, loc + j, 0:HID],
                                    rhs=SS[:],
                                    start=False,
                                    stop=(k == n_ch_t - 1),
                                )
                            else:
                                nc.tensor.matmul(
                                    out=pt[:],
                                    lhsT=SS[:],
                                    rhs=g3[:, loc + j, 0:HID],
                                    start=False,
                                    stop=(k == n_ch_t - 1),
                                )
                            k += 1
                    if layer == 1:
                        nc.scalar.activation(
                            out=h1rT[:, t0 : t0 + tsz], in_=pt[:, :tsz],
                            func=mybir.ActivationFunctionType.Relu,
                            bias=b1t[:],
                        )
                    else:
                        # L = psum + b2 into the batched logits buffer
                        nc.vector.tensor_tensor(
                            out=Lb3[:tsz, t, :], in0=pt[:tsz, :C],
                            in1=b2t[:tsz, :], op=mybir.AluOpType.add,
                        )

        conv(1, h1k3)

        # ---- layer 2 linear: h2_local = relu(h1_agg) @ W2 (zero-padded cols),
        # AllGather in two halves ----
        for t in range(N_TILES):
            t0 = t * P
            tsz = min(P, NT - t0)
            pb = psA.tile([P, HID2], f32, tag="psA")
            nc.tensor.matmul(
                out=pb[:tsz, :], lhsT=h1rT[:, t0 : t0 + tsz], rhs=W2t[:],
                start=True, stop=True,
            )
            nc.scalar.activation(
                out=h2k3[:tsz, t, :], in_=pb[:tsz, 0:HID],
                func=mybir.ActivationFunctionType.Copy,
            )
            if t < TILES_A:
                nc.sync.dma_start(
                    out=hloc[1][0][t0 : t0 + tsz, 0:HID], in_=h2k3[:tsz, t, :]
                )
            else:
                nc.sync.dma_start(
                    out=hloc[1][1][t0 - NA : t0 - NA + tsz, 0:HID],
                    in_=h2k3[:tsz, t, :],
                )
            if t == TILES_A - 1:
                allgather(1, 0)
        allgather(1, 1)

        Lbig = big.tile([P, N_TILES * C], f32)
        Lb3 = Lbig[:].rearrange("p (t c) -> p t c", t=N_TILES)
        if VARIANT == "nocompute":
            nc.vector.memset(Lbig[:], 0.0)

        conv(2, h2k3)

        # ---- batched log_softmax over all tiles ----
        negm = big.tile([P, N_TILES], f32)
        nc.vector.tensor_reduce(
            out=negm[:], in_=Lb3, axis=mybir.AxisListType.X,
            op=mybir.AluOpType.max, negate=True,
        )
        # Lc = L - max (3D broadcast of negm), in place
        Lc = Lbig
        Lc3 = Lb3
        nc.vector.tensor_tensor(
            out=Lc3, in0=Lb3, in1=negm[:].to_broadcast([P, N_TILES, C]),
            op=mybir.AluOpType.add,
        )
        Eb = big.tile([P, N_TILES * C], f32)
        nc.scalar.activation(
            out=Eb[:], in_=Lc[:], func=mybir.ActivationFunctionType.Exp,
        )
        sums = big.tile([P, N_TILES], f32)
        nc.vector.tensor_reduce(
            out=sums[:], in_=Eb[:].rearrange("p (t c) -> p t c", t=N_TILES),
            axis=mybir.AxisListType.X, op=mybir.AluOpType.add,
        )
        lns = big.tile([P, N_TILES], f32)
        nc.scalar.activation(
            out=lns[:], in_=sums[:], func=mybir.ActivationFunctionType.Ln,
        )
        # out = Lc - ln(sum)
        nc.vector.tensor_tensor(
            out=Lc3, in0=Lc3, in1=lns[:].to_broadcast([P, N_TILES, C]),
            op=mybir.AluOpType.subtract,
        )
        # two DMAs: full tiles then the 84-row tail (rows beyond NT are garbage)
        nc.sync.dma_start(
            out=out_p[0 : (N_TILES - 1) * P, :].rearrange("(t p) c -> p t c", t=N_TILES - 1),
            in_=Lc3[:, : N_TILES - 1, :],
        )
        last0 = (N_TILES - 1) * P
        nc.sync.dma_start(
            out=out_p[last0:NT, :], in_=Lc3[: NT - last0, N_TILES - 1, :],
        )

    nc.compile()
    return nc


def kernel(x, src, dst, W1, b1, W2, b2):
    in_maps, plan = _preprocess(x, src, dst, W1, b1, W2, b2)
    nc = _build(plan)
    res = run_bass_kernel_spmd(
        nc, in_maps, list(range(N_CORES)), trace=PROFILE
    )
    _LAST_RESULTS["exec_time_ns"] = getattr(res, "exec_time_ns", None)
    _LAST_RESULTS["profile_json"] = getattr(res, "profile_json", None)
    out = np.concatenate([res.results[c]["out"] for c in range(N_CORES)], axis=0)
    return out.astype(np.float32)
